# revision 34
# baseline (speedup 1.0000x reference)
"""Trainium2 Bass kernel for nn_AugmentPipe (StyleGAN2-ADA style augmentation).

The reference uses a fixed PRNGKey(42), so every random draw is a compile-time
constant.  All image-processing stages (reflect pad, FIR 2x up, affine
grid_sample with flip/rot90/integer-translate geometry, FIR 2x down, per-image
band filter) are linear and separable, so the whole pipeline factors per
sample b into

    out[b,c] = (A_b @ Z_bc @ B_b.T) * mask_b + nm[b,c]

with Z_bc = image[b,c] (transposed when the sample's rot90 is odd), A_b/B_b
fixed 256x256 f32 matrices, mask_b the cutout mask and nm = noise*sigma*mask.
Host code builds the constants once (rel-l2 vs reference ~1.2e-7); the device
kernel runs two 256^3 matmuls per channel on the PE array plus a fused
mask-multiply / noise-add epilogue, data-parallel over 8 NeuronCores
(2 samples per core).
"""
import os
import numpy as np

SYM2 = np.array([-0.12940952255092145, 0.22414386804185735, 0.836516303737469, 0.48296291314469025])
SYM6 = np.array([0.015404109327027373, 0.0034907120842174702, -0.11799011114819057, -0.048311742585633,
                 0.4910559419267466, 0.787641141030194, 0.3379294217276218, -0.07263752278646252,
                 -0.021060292512300564, 0.04472490177066578, 0.0017677118642428036, -0.007800708325034148])

P = 0.4
XFLIP, ROT90, XINT, XINT_MAX = 0.2, 0.2, 0.2, 0.125
IMGFILTER, BANDS, IMGFILTER_STD = 0.2, (1.0, 1.0, 1.0, 1.0), 1.0
NOISE, NOISE_STD, CUTOUT, CUTOUT_SIZE = 0.3, 0.1, 0.6, 0.5

B, C, H, W = 16, 3, 256, 256
NCORES = 8
SPC = B // NCORES  # samples per core

# Per-core DRAM blob layout, as columns of [128, *] partition-major regions.
# xmm (float32r), per sample s at col offset s*2560:
#   AT  (A^T,  2 k-tiles)     cols    0:512   [kk*256+j  <- AT[kk*128+p, j]]
#   BT  (B^T,  2 i-tiles)     cols  512:1024
#   Z   (image, 6 tiles c,hh) cols 1024:2560  [(c*2+hh)*256+w <- x[c, hh*128+p, w]]
# xew (bfloat16), per sample s at col offset s*2048:
#   mask (2 hh-tiles)         cols    0:512
#   nm   (6 tiles c,hh)       cols  512:2048
OUT_COLS = SPC * C * 2 * 256  # 3072

_cache = {}

USE_F32R = os.environ.get("AUG_F32R", "1") == "1"
USE_BF16 = os.environ.get("AUG_BF16", "0") == "1"


# --------------------------------------------------------------------------
# host-side constant factorization (verified against reference: ~1.2e-7 rel)
# --------------------------------------------------------------------------

def _build_constants():
    import jax
    cpu = jax.devices('cpu')[0]
    with jax.default_device(cpu):
        return _build(jax)


def _build(jax):
    import jax.numpy as jnp
    from jax import lax

    def make_fbank():
        lo = SYM2
        hi = lo * (-1.0) ** np.arange(lo.size)
        lo2 = np.convolve(lo, lo[::-1]) / 2
        hi2 = np.convolve(hi, hi[::-1]) / 2
        fb = np.eye(4, 1)
        for i in range(1, 4):
            fb = np.dstack([fb, np.zeros_like(fb)]).reshape(fb.shape[0], -1)[:, :-1]
            fb = np.stack([np.convolve(r, lo2) for r in fb])
            fb[i, (fb.shape[1] - hi2.size) // 2:(fb.shape[1] + hi2.size) // 2] += hi2
        return jnp.asarray(fb, jnp.float32)

    def _mat3(e, Bn):
        cols = [jnp.broadcast_to(jnp.asarray(v, jnp.float32), (Bn,)) for v in e]
        return jnp.stack(cols, -1).reshape(Bn, 3, 3)

    def scale2d(sx, sy, Bn=1):
        return _mat3([sx, 0.0, 0.0, 0.0, sy, 0.0, 0.0, 0.0, 1.0], Bn)

    def rotate2d(theta, Bn):
        c, s = jnp.cos(theta), jnp.sin(theta)
        return _mat3([c, -s, 0.0, s, c, 0.0, 0.0, 0.0, 1.0], Bn)

    def translate2d(tx, ty, Bn=1):
        return _mat3([1.0, 0.0, tx, 0.0, 1.0, ty, 0.0, 0.0, 1.0], Bn)

    def _fir_pass(x, f, up, down, p0, p1, axis):
        if p0 < 0:
            x = lax.slice_in_dim(x, -p0, x.shape[axis], axis=axis); p0 = 0
        if p1 < 0:
            x = lax.slice_in_dim(x, 0, x.shape[axis] + p1, axis=axis); p1 = 0
        Cn = x.shape[1]
        dn = ('NCHW', 'OIHW', 'NCHW')
        if axis == 3:
            w = jnp.tile(f[None, None, None, :], (Cn, 1, 1, 1))
            return lax.conv_general_dilated(x, w, (1, down), ((0, 0), (p0, p1 + up - 1)),
                                            lhs_dilation=(1, up), dimension_numbers=dn, feature_group_count=Cn)
        w = jnp.tile(f[None, None, :, None], (Cn, 1, 1, 1))
        return lax.conv_general_dilated(x, w, (down, 1), ((p0, p1 + up - 1), (0, 0)),
                                        lhs_dilation=(up, 1), dimension_numbers=dn, feature_group_count=Cn)

    def affine_grid(theta, Hh, Ww):
        xs = (2.0 * jnp.arange(Ww, dtype=jnp.float32) + 1.0) / Ww - 1.0
        ys = (2.0 * jnp.arange(Hh, dtype=jnp.float32) + 1.0) / Hh - 1.0
        base = jnp.stack([jnp.broadcast_to(xs[None, :], (Hh, Ww)),
                          jnp.broadcast_to(ys[:, None], (Hh, Ww)),
                          jnp.ones((Hh, Ww), jnp.float32)], -1)
        return jnp.einsum('bkc,hwc->bhwk', theta, base)

    key = jax.random.PRNGKey(42)
    k = jax.random.split(key, 19)
    f_geom = jnp.asarray(SYM6 / SYM6.sum(), jnp.float32)
    fw = f_geom.shape[0]
    Hz_pad = fw // 4
    ones = jnp.ones((B,), jnp.float32)

    i = jnp.floor(jax.random.uniform(k[0], (B,)) * 2)
    i = jnp.where(jax.random.uniform(k[1], (B,)) < XFLIP * P, i, 0.0)
    G = scale2d(1.0 / (1.0 - 2.0 * i), ones, B)

    r = jnp.floor(jax.random.uniform(k[2], (B,)) * 4)
    r = jnp.where(jax.random.uniform(k[3], (B,)) < ROT90 * P, r, 0.0)
    G = G @ rotate2d(np.pi / 2 * r, B)

    t = (jax.random.uniform(k[4], (B, 2)) * 2 - 1) * XINT_MAX
    t = jnp.where(jax.random.uniform(k[5], (B, 1)) < XINT * P, t, 0.0)
    G = G @ translate2d(-jnp.round(t[:, 0] * W), -jnp.round(t[:, 1] * H), B)

    cx, cy = (W - 1) / 2, (H - 1) / 2
    cp = jnp.array([[-cx, -cy, 1], [cx, -cy, 1], [cx, cy, 1], [-cx, cy, 1]], jnp.float32)
    cp = G @ cp.T
    m = cp[:, :2, :].transpose(1, 0, 2).reshape(2, -1)
    m = jnp.concatenate([-m, m], 0).max(axis=1)
    m = np.asarray(m) + np.array([Hz_pad * 2 - cx, Hz_pad * 2 - cy] * 2)
    m = np.clip(m, 0, np.array([W - 1, H - 1] * 2))
    mx0, my0, mx1, my1 = np.ceil(m).astype(int)

    G = translate2d((mx0 - mx1) / 2, (my0 - my1) / 2) @ G
    G = scale2d(2.0, 2.0) @ G @ scale2d(0.5, 0.5)
    G = translate2d(-0.5, -0.5) @ G @ translate2d(0.5, 0.5)
    Hout, Wout = (H + Hz_pad * 2) * 2, (W + Hz_pad * 2) * 2
    Hp, Wp = H + my0 + my1, W + mx0 + mx1
    Hin_u, Win_u = 2 * Hp, 2 * Wp
    G = scale2d(2.0 / Win_u, 2.0 / Hin_u) @ G @ scale2d(Wout / 2.0, Hout / 2.0)

    grid = np.asarray(affine_grid(G[:, :2, :], Hout, Wout))
    px = (grid[..., 0] + np.float32(1.0)) * np.float32(Win_u * 0.5) - np.float32(0.5)
    py = (grid[..., 1] + np.float32(1.0)) * np.float32(Hin_u * 0.5) - np.float32(0.5)

    rr = np.asarray(r).astype(int)
    odd = (rr % 2 == 1)

    def probe_pass(n, up, down, p0, p1, flip, gain):
        f = f_geom
        if not flip:
            f = f[::-1]
        f = (f * np.sqrt(gain)).astype(jnp.float32)
        x = jnp.asarray(np.eye(n, dtype=np.float32)[:, None, None, :])
        y = _fir_pass(x, f, up, down, p0, p1, 3)
        return np.asarray(y)[:, 0, 0, :].T.astype(np.float64)

    def reflect_mat(n, p0, p1):
        idx = np.pad(np.arange(n), (p0, p1), mode='reflect')
        M = np.zeros((n + p0 + p1, n))
        M[np.arange(n + p0 + p1), idx] = 1.0
        return M

    def gather_mat(pos, n_in):
        pos = np.asarray(pos, np.float32)
        n_out = pos.shape[0]
        x0f = np.floor(pos)
        w1 = (pos - x0f).astype(np.float64)
        x0 = x0f.astype(np.int64)
        M = np.zeros((n_out, n_in))
        o = np.arange(n_out)
        v0 = (x0 >= 0) & (x0 < n_in)
        v1 = (x0 + 1 >= 0) & (x0 + 1 < n_in)
        M[o[v0], x0[v0]] += 1.0 - w1[v0]
        M[o[v1], x0[v1] + 1] += w1[v1]
        return M

    Rh = reflect_mat(H, my0, my1)
    Rw = reflect_mat(W, mx0, mx1)
    p0u, p1u = (fw + 1) // 2, (fw - 2) // 2
    Uh = probe_pass(Hp, 2, 1, p0u, p1u, False, 4.0)
    Uw = Uh if Wp == Hp else probe_pass(Wp, 2, 1, p0u, p1u, False, 4.0)
    p0d = -Hz_pad * 2 + (fw - 1) // 2
    p1d = -Hz_pad * 2 + (fw - 2) // 2
    D = probe_pass(Hout, 1, 2, p0d, p1d, True, 1.0)

    fb = make_fbank()
    taps = fb.shape[1]
    expected = jnp.array([10.0, 1.0, 1.0, 1.0], jnp.float32) / 13.0
    g = jnp.ones((B, 4), jnp.float32)
    for bi in range(4):
        ti = jnp.exp2(jax.random.normal(k[6 + 2 * bi], (B,)) * IMGFILTER_STD)
        ti = jnp.where(jax.random.uniform(k[7 + 2 * bi], (B,)) < IMGFILTER * P * BANDS[bi], ti, 1.0)
        tv = jnp.ones((B, 4), jnp.float32).at[:, bi].set(ti)
        tv = tv / jnp.sqrt((expected * tv ** 2).sum(-1, keepdims=True))
        g = g * tv
    Hz_prime = np.asarray(g @ fb, np.float64)
    pb = taps // 2
    Tpad = reflect_mat(H, pb, pb)

    def band_mat(wv):
        T = np.zeros((H, H + 2 * pb))
        for ii in range(H):
            T[ii, ii:ii + taps] = wv
        return T @ Tpad

    h_mid, w_mid = Hout // 2, Wout // 2
    A_list, B_list = [], []
    for b in range(B):
        Fb = band_mat(Hz_prime[b])
        if not odd[b]:
            Gy = gather_mat(py[b, :, w_mid], Hin_u)
            Gx = gather_mat(px[b, h_mid, :], Win_u)
            Amat = Fb @ D @ Gy @ Uh @ Rh
            Bmat = Fb @ D @ Gx @ Uw @ Rw
        else:
            Wx = gather_mat(px[b, :, w_mid], Win_u)
            Wy = gather_mat(py[b, h_mid, :], Hin_u)
            Amat = Fb @ D @ Wx @ Uw @ Rw
            Bmat = Fb @ D @ Wy @ Uh @ Rh
        A_list.append(Amat.astype(np.float32))
        B_list.append(Bmat.astype(np.float32))

    sigma = jnp.abs(jax.random.normal(k[14], (B, 1, 1, 1))) * NOISE_STD
    sigma = jnp.where(jax.random.uniform(k[15], (B, 1, 1, 1)) < NOISE * P, sigma, 0.0)
    noise = np.asarray(jax.random.normal(k[16], (B, C, H, W)) * sigma)

    size = jnp.where(jax.random.uniform(k[17], (B, 1, 1)) < CUTOUT * P, CUTOUT_SIZE, 0.0)
    center = jax.random.uniform(k[18], (B, 2, 1, 1))
    coord_x = (jnp.arange(W, dtype=jnp.float32) + 0.5) / W
    coord_y = (jnp.arange(H, dtype=jnp.float32) + 0.5) / H
    mask_x = jnp.abs(coord_x[None, None, :] - center[:, 0]) >= size / 2
    mask_y = jnp.abs(coord_y[None, :, None] - center[:, 1]) >= size / 2
    mask = np.asarray((mask_x | mask_y).astype(jnp.float32))

    nm = (noise * mask[:, None]).astype(np.float32)

    return {
        'A': np.stack(A_list),
        'Bm': np.stack(B_list),
        'odd': odd,
        'mask': mask.astype(np.float32),
        'nm': nm,
    }


def _get_consts():
    if 'consts' not in _cache:
        _cache['consts'] = _build_constants()
    return _cache['consts']


# --------------------------------------------------------------------------
# device program
# --------------------------------------------------------------------------

def _build_program():
    import concourse.bacc as bacc
    import concourse.mybir as mybir
    from concourse.tile import TileContext

    f32 = mybir.dt.float32
    bf16 = mybir.dt.bfloat16
    if USE_BF16:
        mmdt = mybir.dt.bfloat16
    else:
        mmdt = mybir.dt.float32r if USE_F32R else mybir.dt.float32

    nc = bacc.Bacc(trn_type="TRN2", num_swdge_queues=4)
    xmm = nc.dram_tensor('xmm', [128, SPC * 2560], mmdt, kind='ExternalInput')
    xew = nc.dram_tensor('xew', [128, SPC * 2048], bf16, kind='ExternalInput')
    yout = nc.dram_tensor('out', [128, OUT_COLS], f32, kind='ExternalOutput')

    with TileContext(nc) as tc:
        # Bacc's generate_event_semaphores legalizes multi-wait instructions,
        # so DMAs can be fine-grained.  Input DMAs are split per sample so the
        # first channel's matmuls start as soon as AT|BT|Zc0 lands, while the
        # rest streams in behind.
        with tc.tile_pool(name='persist', bufs=1) as pp, \
             tc.tile_pool(name='work', bufs=6) as wp, \
             tc.tile_pool(name='ps', bufs=3, space='PSUM') as psp:

            # PE warmup on dummy data so HAM is at 2.4GHz when real matmuls
            # arrive (f32 dummies avoid the f32r producer check)
            warm_src = pp.tile([128, 256], f32, tag='warmsrc', name='warmsrc')
            warm_ps = psp.tile([128, 512], f32, tag='warm', name='warm', bufs=1)
            nc.gpsimd.memset(warm_src[:], 0.0)
            for wi in range(11):
                nc.tensor.matmul(warm_ps[:, 0:128], warm_src[:, 0:128],
                                 warm_src[:, 0:128], start=True, stop=True)

            # Coarse per-sample input DMAs on the Sync HWDGE queue (measured
            # fastest): d1 = AT|BT|Zc0 gates the first matmuls, d2 = Zc1|Zc2,
            # d3 = mask|nm (bf16).
            mm_t, ew_t = {}, {}
            for s in range(SPC):
                o = s * 2560
                mm_t[s] = pp.tile([128, 2560], mmdt, tag=f'mm{s}', name=f'mm{s}')
                ew_t[s] = pp.tile([128, 2048], bf16, tag=f'ew{s}', name=f'ew{s}')
                # d1 = AT|BT|Zc0 gates the first matmuls; d2 = Zc1|Zc2;
                # d3 = mask|nm (bf16); all on the Sync HWDGE queue (measured
                # fastest across sync/gpsimd splits)
                nc.sync.dma_start(out=mm_t[s][:, 0:1536], in_=xmm[:, o:o + 1536])
                nc.sync.dma_start(out=mm_t[s][:, 1536:2560], in_=xmm[:, o + 1536:o + 2560])
                nc.sync.dma_start(out=ew_t[s][:], in_=xew[:, s * 2048:(s + 1) * 2048])
            at_t = {s: mm_t[s][:, 0:512] for s in range(SPC)}
            bt_t = {s: mm_t[s][:, 512:1024] for s in range(SPC)}

            def zcol(c, kk):
                return 1024 + (c * 2 + kk) * 256
            mk_t = {s: ew_t[s][:, 0:512] for s in range(SPC)}
            nm_t = {s: ew_t[s][:, 512:2048] for s in range(SPC)}

            osamp = {s: pp.tile([128, C * 512], f32, tag=f'os{s}', name=f'os{s}')
                     for s in range(SPC)}

            def stage1(s, c):
                """S = Z^T @ AT into one single-bank [128,512] PSUM tile,
                then one CAST to f32r SBUF for stage 2's stationary operand."""
                s_sb = wp.tile([128, 512], mmdt, tag='s_sb')
                s_ps = psp.tile([128, 512], f32, tag='s_ps')
                for iblk in range(2):
                    for kk in range(2):
                        zb = zcol(c, kk)
                        lhsT = mm_t[s][:, zb + iblk * 128:zb + iblk * 128 + 128]
                        rhs = at_t[s][:, kk * 256:(kk + 1) * 256]
                        nc.tensor.matmul(s_ps[:, iblk * 256:(iblk + 1) * 256],
                                         lhsT, rhs,
                                         start=(kk == 0), stop=(kk == 1))
                nc.vector.tensor_copy(out=s_sb[:], in_=s_ps[:])
                return s_sb

            def stage2(s, c, s_sb):
                """O = S^T @ BT (single-bank PSUM), epilogue O*mask + nm."""
                idx = s * C + c
                o_ps = psp.tile([128, 512], f32, tag='o_ps')
                for pblk in range(2):
                    for jblk in range(2):
                        lhsT = s_sb[:, jblk * 256 + pblk * 128:jblk * 256 + pblk * 128 + 128]
                        rhs = bt_t[s][:, jblk * 256:(jblk + 1) * 256]
                        nc.tensor.matmul(o_ps[:, pblk * 256:(pblk + 1) * 256],
                                         lhsT, rhs,
                                         start=(jblk == 0), stop=(jblk == 1))
                dst = osamp[s][:, c * 512:(c + 1) * 512]
                nc.vector.tensor_mul(out=dst, in0=o_ps[:], in1=mk_t[s][:, 0:512])
                # GpSimd add relieves the DVE mid-kernel; the last two
                # channels' adds go to the then-idle DVE so the tail chain
                # is short (GpSimd adds are ~2x slower and serialized)
                add_eng = nc.gpsimd if idx < 4 else nc.vector
                add_eng.tensor_add(out=dst, in0=dst,
                                   in1=nm_t[s][:, c * 512:(c + 1) * 512])
                nc.sync.dma_start(out=yout[:, idx * 512:idx * 512 + 512],
                                  in_=osamp[s][:, c * 512:(c + 1) * 512])

            # software-pipelined emission: stage1(next) before stage2(cur)
            chans = [(s, c) for s in range(SPC) for c in range(C)]
            prev = None
            for (s, c) in chans:
                s_sb = stage1(s, c)
                if prev is not None:
                    stage2(*prev)
                prev = (s, c, s_sb)
            stage2(*prev)

    nc.compile()
    return nc


def _get_program():
    if 'nc' not in _cache:
        _cache['nc'] = _build_program()
    return _cache['nc']


# --------------------------------------------------------------------------
# host packing / unpacking
# --------------------------------------------------------------------------

def _pack_blobs(image, cst):
    A, Bm, odd, mask, nm = cst['A'], cst['Bm'], cst['odd'], cst['mask'], cst['nm']
    blobs = []
    for core in range(NCORES):
        import ml_dtypes
        mmnp = ml_dtypes.bfloat16 if USE_BF16 else np.float32
        bmm = np.empty((128, SPC * 2560), mmnp)
        bew = np.empty((128, SPC * 2048), ml_dtypes.bfloat16)
        for s in range(SPC):
            g = core * SPC + s
            o = s * 2560
            e = s * 2048
            x = image[g] if not odd[g] else np.ascontiguousarray(image[g].transpose(0, 2, 1))
            bmm[:, o:o + 512] = A[g].T.reshape(2, 128, 256).transpose(1, 0, 2).reshape(128, 512)
            bmm[:, o + 512:o + 1024] = Bm[g].T.reshape(2, 128, 256).transpose(1, 0, 2).reshape(128, 512)
            bmm[:, o + 1024:o + 2560] = x.reshape(3, 2, 128, 256).transpose(2, 0, 1, 3).reshape(128, 1536)
            bew[:, e:e + 512] = mask[g].reshape(2, 128, 256).transpose(1, 0, 2).reshape(128, 512)
            bew[:, e + 512:e + 2048] = nm[g].reshape(3, 2, 128, 256).transpose(2, 0, 1, 3).reshape(128, 1536)
        blobs.append({'xmm': bmm, 'xew': bew})
    return blobs


def _unpack_out(results):
    out = np.empty((B, C, H, W), np.float32)
    for core in range(NCORES):
        blob = results[core]['out']  # [128, 3072]
        o2 = blob.reshape(128, SPC, C, 2, 256).transpose(1, 2, 3, 0, 4).reshape(SPC, C, H, W)
        out[core * SPC:(core + 1) * SPC] = o2
    return out


# --------------------------------------------------------------------------
# entry point
# --------------------------------------------------------------------------

def kernel(image, _run_kwargs=None):
    from concourse.bass_utils import run_bass_kernel_spmd

    image = np.ascontiguousarray(np.asarray(image), dtype=np.float32)
    cst = _get_consts()
    nc = _get_program()
    in_maps = _pack_blobs(image, cst)
    res = run_bass_kernel_spmd(nc, in_maps, core_ids=list(range(NCORES)),
                               **(_run_kwargs or {}))
    if _run_kwargs is not None:
        _cache['last_results'] = res
    return _unpack_out(res.results)


# revision 36
# speedup vs baseline: 1.0033x; 1.0033x over previous
"""Trainium2 Bass kernel for nn_AugmentPipe (StyleGAN2-ADA style augmentation).

The reference uses a fixed PRNGKey(42), so every random draw is a compile-time
constant.  All image-processing stages (reflect pad, FIR 2x up, affine
grid_sample with flip/rot90/integer-translate geometry, FIR 2x down, per-image
band filter) are linear and separable, so the whole pipeline factors per
sample b into

    out[b,c] = (A_b @ Z_bc @ B_b.T) * mask_b + nm[b,c]

with Z_bc = image[b,c] (transposed when the sample's rot90 is odd), A_b/B_b
fixed 256x256 f32 matrices, mask_b the cutout mask and nm = noise*sigma*mask.
Host code builds the constants once (rel-l2 vs reference ~1.2e-7); the device
kernel runs two 256^3 matmuls per channel on the PE array plus a fused
mask-multiply / noise-add epilogue, data-parallel over 8 NeuronCores
(2 samples per core).
"""
import os
import numpy as np

SYM2 = np.array([-0.12940952255092145, 0.22414386804185735, 0.836516303737469, 0.48296291314469025])
SYM6 = np.array([0.015404109327027373, 0.0034907120842174702, -0.11799011114819057, -0.048311742585633,
                 0.4910559419267466, 0.787641141030194, 0.3379294217276218, -0.07263752278646252,
                 -0.021060292512300564, 0.04472490177066578, 0.0017677118642428036, -0.007800708325034148])

P = 0.4
XFLIP, ROT90, XINT, XINT_MAX = 0.2, 0.2, 0.2, 0.125
IMGFILTER, BANDS, IMGFILTER_STD = 0.2, (1.0, 1.0, 1.0, 1.0), 1.0
NOISE, NOISE_STD, CUTOUT, CUTOUT_SIZE = 0.3, 0.1, 0.6, 0.5

B, C, H, W = 16, 3, 256, 256
NCORES = 8
SPC = B // NCORES  # samples per core

# Per-core DRAM blob layout, as columns of [128, *] partition-major regions.
# xmm (float32r), per sample s at col offset s*2560:
#   AT  (A^T,  2 k-tiles)     cols    0:512   [kk*256+j  <- AT[kk*128+p, j]]
#   BT  (B^T,  2 i-tiles)     cols  512:1024
#   Z   (image, 6 tiles c,hh) cols 1024:2560  [(c*2+hh)*256+w <- x[c, hh*128+p, w]]
# xew (bfloat16), per sample s at col offset s*2048:
#   mask (2 hh-tiles)         cols    0:512
#   nm   (6 tiles c,hh)       cols  512:2048
OUT_COLS = SPC * C * 2 * 256  # 3072

_cache = {}

USE_F32R = os.environ.get("AUG_F32R", "1") == "1"
USE_BF16 = os.environ.get("AUG_BF16", "0") == "1"


# --------------------------------------------------------------------------
# host-side constant factorization (verified against reference: ~1.2e-7 rel)
# --------------------------------------------------------------------------

def _build_constants():
    import jax
    cpu = jax.devices('cpu')[0]
    with jax.default_device(cpu):
        return _build(jax)


def _build(jax):
    import jax.numpy as jnp
    from jax import lax

    def make_fbank():
        lo = SYM2
        hi = lo * (-1.0) ** np.arange(lo.size)
        lo2 = np.convolve(lo, lo[::-1]) / 2
        hi2 = np.convolve(hi, hi[::-1]) / 2
        fb = np.eye(4, 1)
        for i in range(1, 4):
            fb = np.dstack([fb, np.zeros_like(fb)]).reshape(fb.shape[0], -1)[:, :-1]
            fb = np.stack([np.convolve(r, lo2) for r in fb])
            fb[i, (fb.shape[1] - hi2.size) // 2:(fb.shape[1] + hi2.size) // 2] += hi2
        return jnp.asarray(fb, jnp.float32)

    def _mat3(e, Bn):
        cols = [jnp.broadcast_to(jnp.asarray(v, jnp.float32), (Bn,)) for v in e]
        return jnp.stack(cols, -1).reshape(Bn, 3, 3)

    def scale2d(sx, sy, Bn=1):
        return _mat3([sx, 0.0, 0.0, 0.0, sy, 0.0, 0.0, 0.0, 1.0], Bn)

    def rotate2d(theta, Bn):
        c, s = jnp.cos(theta), jnp.sin(theta)
        return _mat3([c, -s, 0.0, s, c, 0.0, 0.0, 0.0, 1.0], Bn)

    def translate2d(tx, ty, Bn=1):
        return _mat3([1.0, 0.0, tx, 0.0, 1.0, ty, 0.0, 0.0, 1.0], Bn)

    def _fir_pass(x, f, up, down, p0, p1, axis):
        if p0 < 0:
            x = lax.slice_in_dim(x, -p0, x.shape[axis], axis=axis); p0 = 0
        if p1 < 0:
            x = lax.slice_in_dim(x, 0, x.shape[axis] + p1, axis=axis); p1 = 0
        Cn = x.shape[1]
        dn = ('NCHW', 'OIHW', 'NCHW')
        if axis == 3:
            w = jnp.tile(f[None, None, None, :], (Cn, 1, 1, 1))
            return lax.conv_general_dilated(x, w, (1, down), ((0, 0), (p0, p1 + up - 1)),
                                            lhs_dilation=(1, up), dimension_numbers=dn, feature_group_count=Cn)
        w = jnp.tile(f[None, None, :, None], (Cn, 1, 1, 1))
        return lax.conv_general_dilated(x, w, (down, 1), ((p0, p1 + up - 1), (0, 0)),
                                        lhs_dilation=(up, 1), dimension_numbers=dn, feature_group_count=Cn)

    def affine_grid(theta, Hh, Ww):
        xs = (2.0 * jnp.arange(Ww, dtype=jnp.float32) + 1.0) / Ww - 1.0
        ys = (2.0 * jnp.arange(Hh, dtype=jnp.float32) + 1.0) / Hh - 1.0
        base = jnp.stack([jnp.broadcast_to(xs[None, :], (Hh, Ww)),
                          jnp.broadcast_to(ys[:, None], (Hh, Ww)),
                          jnp.ones((Hh, Ww), jnp.float32)], -1)
        return jnp.einsum('bkc,hwc->bhwk', theta, base)

    key = jax.random.PRNGKey(42)
    k = jax.random.split(key, 19)
    f_geom = jnp.asarray(SYM6 / SYM6.sum(), jnp.float32)
    fw = f_geom.shape[0]
    Hz_pad = fw // 4
    ones = jnp.ones((B,), jnp.float32)

    i = jnp.floor(jax.random.uniform(k[0], (B,)) * 2)
    i = jnp.where(jax.random.uniform(k[1], (B,)) < XFLIP * P, i, 0.0)
    G = scale2d(1.0 / (1.0 - 2.0 * i), ones, B)

    r = jnp.floor(jax.random.uniform(k[2], (B,)) * 4)
    r = jnp.where(jax.random.uniform(k[3], (B,)) < ROT90 * P, r, 0.0)
    G = G @ rotate2d(np.pi / 2 * r, B)

    t = (jax.random.uniform(k[4], (B, 2)) * 2 - 1) * XINT_MAX
    t = jnp.where(jax.random.uniform(k[5], (B, 1)) < XINT * P, t, 0.0)
    G = G @ translate2d(-jnp.round(t[:, 0] * W), -jnp.round(t[:, 1] * H), B)

    cx, cy = (W - 1) / 2, (H - 1) / 2
    cp = jnp.array([[-cx, -cy, 1], [cx, -cy, 1], [cx, cy, 1], [-cx, cy, 1]], jnp.float32)
    cp = G @ cp.T
    m = cp[:, :2, :].transpose(1, 0, 2).reshape(2, -1)
    m = jnp.concatenate([-m, m], 0).max(axis=1)
    m = np.asarray(m) + np.array([Hz_pad * 2 - cx, Hz_pad * 2 - cy] * 2)
    m = np.clip(m, 0, np.array([W - 1, H - 1] * 2))
    mx0, my0, mx1, my1 = np.ceil(m).astype(int)

    G = translate2d((mx0 - mx1) / 2, (my0 - my1) / 2) @ G
    G = scale2d(2.0, 2.0) @ G @ scale2d(0.5, 0.5)
    G = translate2d(-0.5, -0.5) @ G @ translate2d(0.5, 0.5)
    Hout, Wout = (H + Hz_pad * 2) * 2, (W + Hz_pad * 2) * 2
    Hp, Wp = H + my0 + my1, W + mx0 + mx1
    Hin_u, Win_u = 2 * Hp, 2 * Wp
    G = scale2d(2.0 / Win_u, 2.0 / Hin_u) @ G @ scale2d(Wout / 2.0, Hout / 2.0)

    grid = np.asarray(affine_grid(G[:, :2, :], Hout, Wout))
    px = (grid[..., 0] + np.float32(1.0)) * np.float32(Win_u * 0.5) - np.float32(0.5)
    py = (grid[..., 1] + np.float32(1.0)) * np.float32(Hin_u * 0.5) - np.float32(0.5)

    rr = np.asarray(r).astype(int)
    odd = (rr % 2 == 1)

    def probe_pass(n, up, down, p0, p1, flip, gain):
        f = f_geom
        if not flip:
            f = f[::-1]
        f = (f * np.sqrt(gain)).astype(jnp.float32)
        x = jnp.asarray(np.eye(n, dtype=np.float32)[:, None, None, :])
        y = _fir_pass(x, f, up, down, p0, p1, 3)
        return np.asarray(y)[:, 0, 0, :].T.astype(np.float64)

    def reflect_mat(n, p0, p1):
        idx = np.pad(np.arange(n), (p0, p1), mode='reflect')
        M = np.zeros((n + p0 + p1, n))
        M[np.arange(n + p0 + p1), idx] = 1.0
        return M

    def gather_mat(pos, n_in):
        pos = np.asarray(pos, np.float32)
        n_out = pos.shape[0]
        x0f = np.floor(pos)
        w1 = (pos - x0f).astype(np.float64)
        x0 = x0f.astype(np.int64)
        M = np.zeros((n_out, n_in))
        o = np.arange(n_out)
        v0 = (x0 >= 0) & (x0 < n_in)
        v1 = (x0 + 1 >= 0) & (x0 + 1 < n_in)
        M[o[v0], x0[v0]] += 1.0 - w1[v0]
        M[o[v1], x0[v1] + 1] += w1[v1]
        return M

    Rh = reflect_mat(H, my0, my1)
    Rw = reflect_mat(W, mx0, mx1)
    p0u, p1u = (fw + 1) // 2, (fw - 2) // 2
    Uh = probe_pass(Hp, 2, 1, p0u, p1u, False, 4.0)
    Uw = Uh if Wp == Hp else probe_pass(Wp, 2, 1, p0u, p1u, False, 4.0)
    p0d = -Hz_pad * 2 + (fw - 1) // 2
    p1d = -Hz_pad * 2 + (fw - 2) // 2
    D = probe_pass(Hout, 1, 2, p0d, p1d, True, 1.0)

    fb = make_fbank()
    taps = fb.shape[1]
    expected = jnp.array([10.0, 1.0, 1.0, 1.0], jnp.float32) / 13.0
    g = jnp.ones((B, 4), jnp.float32)
    for bi in range(4):
        ti = jnp.exp2(jax.random.normal(k[6 + 2 * bi], (B,)) * IMGFILTER_STD)
        ti = jnp.where(jax.random.uniform(k[7 + 2 * bi], (B,)) < IMGFILTER * P * BANDS[bi], ti, 1.0)
        tv = jnp.ones((B, 4), jnp.float32).at[:, bi].set(ti)
        tv = tv / jnp.sqrt((expected * tv ** 2).sum(-1, keepdims=True))
        g = g * tv
    Hz_prime = np.asarray(g @ fb, np.float64)
    pb = taps // 2
    Tpad = reflect_mat(H, pb, pb)

    def band_mat(wv):
        T = np.zeros((H, H + 2 * pb))
        for ii in range(H):
            T[ii, ii:ii + taps] = wv
        return T @ Tpad

    h_mid, w_mid = Hout // 2, Wout // 2
    A_list, B_list = [], []
    for b in range(B):
        Fb = band_mat(Hz_prime[b])
        if not odd[b]:
            Gy = gather_mat(py[b, :, w_mid], Hin_u)
            Gx = gather_mat(px[b, h_mid, :], Win_u)
            Amat = Fb @ D @ Gy @ Uh @ Rh
            Bmat = Fb @ D @ Gx @ Uw @ Rw
        else:
            Wx = gather_mat(px[b, :, w_mid], Win_u)
            Wy = gather_mat(py[b, h_mid, :], Hin_u)
            Amat = Fb @ D @ Wx @ Uw @ Rw
            Bmat = Fb @ D @ Wy @ Uh @ Rh
        A_list.append(Amat.astype(np.float32))
        B_list.append(Bmat.astype(np.float32))

    sigma = jnp.abs(jax.random.normal(k[14], (B, 1, 1, 1))) * NOISE_STD
    sigma = jnp.where(jax.random.uniform(k[15], (B, 1, 1, 1)) < NOISE * P, sigma, 0.0)
    noise = np.asarray(jax.random.normal(k[16], (B, C, H, W)) * sigma)

    size = jnp.where(jax.random.uniform(k[17], (B, 1, 1)) < CUTOUT * P, CUTOUT_SIZE, 0.0)
    center = jax.random.uniform(k[18], (B, 2, 1, 1))
    coord_x = (jnp.arange(W, dtype=jnp.float32) + 0.5) / W
    coord_y = (jnp.arange(H, dtype=jnp.float32) + 0.5) / H
    mask_x = jnp.abs(coord_x[None, None, :] - center[:, 0]) >= size / 2
    mask_y = jnp.abs(coord_y[None, :, None] - center[:, 1]) >= size / 2
    mask = np.asarray((mask_x | mask_y).astype(jnp.float32))

    nm = (noise * mask[:, None]).astype(np.float32)

    return {
        'A': np.stack(A_list),
        'Bm': np.stack(B_list),
        'odd': odd,
        'mask': mask.astype(np.float32),
        'nm': nm,
    }


def _get_consts():
    if 'consts' not in _cache:
        _cache['consts'] = _build_constants()
    return _cache['consts']


# --------------------------------------------------------------------------
# device program
# --------------------------------------------------------------------------

def _build_program():
    import concourse.bacc as bacc
    import concourse.mybir as mybir
    from concourse.tile import TileContext

    f32 = mybir.dt.float32
    bf16 = mybir.dt.bfloat16
    if USE_BF16:
        mmdt = mybir.dt.bfloat16
    else:
        mmdt = mybir.dt.float32r if USE_F32R else mybir.dt.float32

    nc = bacc.Bacc(trn_type="TRN2", num_swdge_queues=4)
    xmm = nc.dram_tensor('xmm', [128, SPC * 2560], mmdt, kind='ExternalInput')
    xew = nc.dram_tensor('xew', [128, SPC * 2048], bf16, kind='ExternalInput')
    yout = nc.dram_tensor('out', [128, OUT_COLS], f32, kind='ExternalOutput')

    with TileContext(nc) as tc:
        # Bacc's generate_event_semaphores legalizes multi-wait instructions,
        # so DMAs can be fine-grained.  Input DMAs are split per sample so the
        # first channel's matmuls start as soon as AT|BT|Zc0 lands, while the
        # rest streams in behind.
        with tc.tile_pool(name='persist', bufs=1) as pp, \
             tc.tile_pool(name='work', bufs=6) as wp, \
             tc.tile_pool(name='ps', bufs=3, space='PSUM') as psp:

            # PE warmup on dummy data so HAM is at 2.4GHz when real matmuls
            # arrive (f32 dummies avoid the f32r producer check)
            warm_src = pp.tile([128, 256], f32, tag='warmsrc', name='warmsrc')
            warm_ps = psp.tile([128, 512], f32, tag='warm', name='warm', bufs=1)
            nc.gpsimd.memset(warm_src[:], 0.0)
            for wi in range(11):
                nc.tensor.matmul(warm_ps[:, 0:128], warm_src[:, 0:128],
                                 warm_src[:, 0:128], start=True, stop=True)

            # Coarse per-sample input DMAs on the Sync HWDGE queue (measured
            # fastest): d1 = AT|BT|Zc0 gates the first matmuls, d2 = Zc1|Zc2,
            # d3 = mask|nm (bf16).
            mm_t, ew_t = {}, {}
            for s in range(SPC):
                o = s * 2560
                mm_t[s] = pp.tile([128, 2560], mmdt, tag=f'mm{s}', name=f'mm{s}')
                ew_t[s] = pp.tile([128, 2048], bf16, tag=f'ew{s}', name=f'ew{s}')
                # d1 = AT|BT|Zc0 gates the first matmuls; d2 = Zc1|Zc2;
                # d3 = mask|nm (bf16); all on the Sync HWDGE queue (measured
                # fastest across sync/gpsimd splits)
                nc.sync.dma_start(out=mm_t[s][:, 0:1536], in_=xmm[:, o:o + 1536])
                nc.sync.dma_start(out=mm_t[s][:, 1536:2560], in_=xmm[:, o + 1536:o + 2560])
                nc.sync.dma_start(out=ew_t[s][:], in_=xew[:, s * 2048:(s + 1) * 2048])
            at_t = {s: mm_t[s][:, 0:512] for s in range(SPC)}
            bt_t = {s: mm_t[s][:, 512:1024] for s in range(SPC)}

            def zcol(c, kk):
                return 1024 + (c * 2 + kk) * 256
            mk_t = {s: ew_t[s][:, 0:512] for s in range(SPC)}
            nm_t = {s: ew_t[s][:, 512:2048] for s in range(SPC)}

            osamp = {s: pp.tile([128, C * 512], f32, tag=f'os{s}', name=f'os{s}')
                     for s in range(SPC)}

            def stage1(s, c):
                """S = Z^T @ AT into one single-bank [128,512] PSUM tile,
                then one CAST to f32r SBUF for stage 2's stationary operand."""
                s_sb = wp.tile([128, 512], mmdt, tag='s_sb')
                s_ps = psp.tile([128, 512], f32, tag='s_ps')
                for iblk in range(2):
                    for kk in range(2):
                        zb = zcol(c, kk)
                        lhsT = mm_t[s][:, zb + iblk * 128:zb + iblk * 128 + 128]
                        rhs = at_t[s][:, kk * 256:(kk + 1) * 256]
                        nc.tensor.matmul(s_ps[:, iblk * 256:(iblk + 1) * 256],
                                         lhsT, rhs,
                                         start=(kk == 0), stop=(kk == 1))
                nc.vector.tensor_copy(out=s_sb[:], in_=s_ps[:])
                return s_sb

            def stage2(s, c, s_sb):
                """O = S^T @ BT (single-bank PSUM), epilogue O*mask + nm."""
                idx = s * C + c
                o_ps = psp.tile([128, 512], f32, tag='o_ps')
                for pblk in range(2):
                    for jblk in range(2):
                        lhsT = s_sb[:, jblk * 256 + pblk * 128:jblk * 256 + pblk * 128 + 128]
                        rhs = bt_t[s][:, jblk * 256:(jblk + 1) * 256]
                        nc.tensor.matmul(o_ps[:, pblk * 256:(pblk + 1) * 256],
                                         lhsT, rhs,
                                         start=(jblk == 0), stop=(jblk == 1))
                dst = osamp[s][:, c * 512:(c + 1) * 512]
                nc.vector.tensor_mul(out=dst, in0=o_ps[:], in1=mk_t[s][:, 0:512])
                # GpSimd add relieves the DVE mid-kernel; the last two
                # channels' adds go to the then-idle DVE so the tail chain
                # is short (GpSimd adds are ~2x slower and serialized)
                add_eng = nc.gpsimd if idx < 4 else nc.vector
                add_eng.tensor_add(out=dst, in0=dst,
                                   in1=nm_t[s][:, c * 512:(c + 1) * 512])
                nc.sync.dma_start(out=yout[:, idx * 512:idx * 512 + 512],
                                  in_=osamp[s][:, c * 512:(c + 1) * 512])

            # software-pipelined emission: stage1(next) before stage2(cur)
            chans = [(s, c) for s in range(SPC) for c in range(C)]
            prev = None
            for (s, c) in chans:
                s_sb = stage1(s, c)
                if prev is not None:
                    stage2(*prev)
                prev = (s, c, s_sb)
            stage2(*prev)

    nc.compile()
    return nc


def _get_program():
    if 'nc' not in _cache:
        _cache['nc'] = _build_program()
    return _cache['nc']


# --------------------------------------------------------------------------
# host packing / unpacking
# --------------------------------------------------------------------------

def _pack_blobs(image, cst):
    A, Bm, odd, mask, nm = cst['A'], cst['Bm'], cst['odd'], cst['mask'], cst['nm']
    blobs = []
    for core in range(NCORES):
        import ml_dtypes
        mmnp = ml_dtypes.bfloat16 if USE_BF16 else np.float32
        bmm = np.empty((128, SPC * 2560), mmnp)
        bew = np.empty((128, SPC * 2048), ml_dtypes.bfloat16)
        for s in range(SPC):
            g = core * SPC + s
            o = s * 2560
            e = s * 2048
            x = image[g] if not odd[g] else np.ascontiguousarray(image[g].transpose(0, 2, 1))
            bmm[:, o:o + 512] = A[g].T.reshape(2, 128, 256).transpose(1, 0, 2).reshape(128, 512)
            bmm[:, o + 512:o + 1024] = Bm[g].T.reshape(2, 128, 256).transpose(1, 0, 2).reshape(128, 512)
            bmm[:, o + 1024:o + 2560] = x.reshape(3, 2, 128, 256).transpose(2, 0, 1, 3).reshape(128, 1536)
            bew[:, e:e + 512] = mask[g].reshape(2, 128, 256).transpose(1, 0, 2).reshape(128, 512)
            bew[:, e + 512:e + 2048] = nm[g].reshape(3, 2, 128, 256).transpose(2, 0, 1, 3).reshape(128, 1536)
        blobs.append({'xmm': bmm, 'xew': bew})
    return blobs


def _unpack_out(results):
    out = np.empty((B, C, H, W), np.float32)
    for core in range(NCORES):
        blob = results[core]['out']  # [128, 3072]
        o2 = blob.reshape(128, SPC, C, 2, 256).transpose(1, 2, 3, 0, 4).reshape(SPC, C, H, W)
        out[core * SPC:(core + 1) * SPC] = o2
    return out


# --------------------------------------------------------------------------
# entry point
# --------------------------------------------------------------------------

def kernel(image, _run_kwargs=None):
    from concourse.bass_utils import run_bass_kernel_spmd

    image = np.ascontiguousarray(np.asarray(image), dtype=np.float32)
    cst = _get_consts()
    nc = _get_program()
    in_maps = _pack_blobs(image, cst)
    res = run_bass_kernel_spmd(nc, in_maps, core_ids=list(range(NCORES)),
                               **(_run_kwargs or {}))
    if _run_kwargs is not None:
        _cache['last_results'] = res
    return _unpack_out(res.results)
